# revision 3
# baseline (speedup 1.0000x reference)
"""Trainium2 Bass kernel for a separable 2-D db4 DWT (pywt 'symmetric' mode).

Input  x: [8, 512, 512, 16] f32, dec_lo: [8] f32.
Output (aa, ad, da, dd): each [8, 259, 259, 16] f32.

Sharding: pure data parallel — batch i runs on core i (8 cores).

Per-core algorithm (x1: [512, 512, 16]):
  Both DWT stages are banded matmuls on the tensor engine with the *data* as
  the stationary operand, so the stage-1 result comes out transposed (W on
  partitions) exactly as stage 2 needs it:

    stage 1:  out1[w, (s,ho)]  = sum_k Xp[k, w]   * B[k, (s,ho)]    (contract H)
    stage 2:  out2[(s,ho), (s',wo)] = sum_k T1[k, (s,ho)] * B[k, (s',wo)]  (contract W)

  B is a [128, 120] banded filter matrix: B[k, s*60+j] = f_s_rev[k - (2j+1)],
  holding both the low- and highpass filters for 60 output positions; a K=128
  input window yields 60 outputs of both subbands in one matmul. Output
  positions tile in blocks of 60 (window stride 120), 5 tiles covering 259.
  Symmetric padding (7 each side) is materialized in SBUF: the H pad via
  mirrored-row DMAs, the W pad via small on-chip column copies.
"""

from contextlib import ExitStack

import numpy as np

B_, H_, W_, C_ = 8, 512, 512, 16
L, PAD = 8, 7
OUT = (H_ + L - 1) // 2            # 259
T = 60                             # output positions per tile
NT = (OUT + T - 1) // T            # 5
WSTRIDE = 2 * T                    # 120 (input window stride)
NPAD = H_ + 2 * PAD                # 526
N_CORES = 8
BANDW = 2 * T                      # 120 band columns
CG = 4                             # channels packed per PSUM bank


def _tile_params(t):
    cnt = min(T, OUT - T * t)      # output positions in tile t
    k0 = WSTRIDE * t               # padded-axis window start
    kw = min(128, NPAD - k0)       # window size (contraction K)
    return cnt, k0, kw


def band_matrix(dec_lo):
    dec_lo = np.asarray(dec_lo, np.float32)
    signs = np.where(np.arange(L) % 2 == 0, -1.0, 1.0).astype(np.float32)
    dec_hi = signs * dec_lo[::-1]
    lo_rev = dec_lo[::-1].copy()
    hi_rev = dec_hi[::-1].copy()
    B = np.zeros((128, BANDW), np.float32)
    for s, f in enumerate((lo_rev, hi_rev)):
        for j in range(T):
            for m in range(L):
                k = 2 * j + 1 + m
                if k < 128:
                    B[k, s * T + j] = f[m]
    return B


def build_nc():
    import concourse.bacc as bacc
    import concourse.mybir as mybir
    import concourse.tile as tile

    f32 = mybir.dt.float32
    nc = bacc.Bacc("TRN2", debug=False, num_devices=N_CORES)
    x = nc.dram_tensor("x", [H_, W_, C_], f32, kind="ExternalInput").ap()
    band = nc.dram_tensor("band", [128, BANDW], f32, kind="ExternalInput").ap()
    out_names = {(0, 0): "aa", (0, 1): "ad", (1, 0): "da", (1, 1): "dd"}
    outs = {
        k: nc.dram_tensor(v, [OUT, OUT, C_], f32, kind="ExternalOutput").ap()
        for k, v in out_names.items()
    }

    evac_engines = [None, None]  # filled after nc exists

    with tile.TileContext(nc) as tc, ExitStack() as ctx:
        xp_pool = ctx.enter_context(tc.tile_pool(name="xp", bufs=2))
        band_pool = ctx.enter_context(tc.tile_pool(name="bandp", bufs=1))
        t1_pool = ctx.enter_context(tc.tile_pool(name="t1", bufs=2))
        stg_pool = ctx.enter_context(tc.tile_pool(name="stg", bufs=3))
        ps1_pool = ctx.enter_context(tc.tile_pool(name="ps1", bufs=3, space="PSUM"))
        ps2_pool = ctx.enter_context(tc.tile_pool(name="ps2", bufs=3, space="PSUM"))

        bt = band_pool.tile([128, BANDW], f32)
        nc.sync.dma_start(bt[:], band[:])

        def _evac(i, dst, src):
            if i % 2 == 0:
                nc.vector.tensor_copy(dst, src)
            else:
                nc.scalar.copy(dst, src)

        n_evac = 0

        def band_ap(kw, cnt):
            # [kw, 2, cnt] free AP over the band tile: (s: stride T, j: 1)
            bap = bt[0:kw, :].rearrange("k (s j) -> k s j", s=2)
            return bap[:, :, 0:cnt]

        for t in range(NT):
            cnt, k0, kw = _tile_params(t)
            ncols = 2 * cnt
            xp = xp_pool.tile([128, NPAD * C_], f32, tag="xp")

            # ---- load H-window (rows k0..k0+kw of padded H) with W pad ----
            # interior rows: partition p <-> x row k0 + p - PAD
            p_lo = PAD - k0 if k0 < PAD else 0          # first interior partition
            hx_lo = max(0, k0 - PAD)
            p_hi = min(kw, H_ + PAD - k0)               # one past last interior
            hx_hi = hx_lo + (p_hi - p_lo)
            nc.sync.dma_start(
                xp[p_lo:p_hi, PAD * C_:(PAD + W_) * C_],
                x[hx_lo:hx_hi].rearrange("h w c -> h (w c)"),
            )
            for p in range(0, p_lo):                    # top H mirror (t == 0)
                nc.sync.dma_start(
                    xp[p:p + 1, PAD * C_:(PAD + W_) * C_],
                    x[6 - p - k0:7 - p - k0].rearrange("h w c -> h (w c)"),
                )
            for p in range(p_hi, kw):                   # bottom H mirror (last t)
                hx = 2 * H_ - 1 + PAD - k0 - p          # 1030 - (k0 + p) for H=512
                nc.sync.dma_start(
                    xp[p:p + 1, PAD * C_:(PAD + W_) * C_],
                    x[hx:hx + 1].rearrange("h w c -> h (w c)"),
                )
            # W mirror columns, copied on-chip from the loaded body
            for j in range(PAD):
                nc.gpsimd.tensor_copy(
                    xp[0:kw, j * C_:(j + 1) * C_],
                    xp[0:kw, (2 * PAD - 1 - j) * C_:(2 * PAD - j) * C_],
                )
                dst = NPAD - PAD + j
                src = PAD + W_ - 1 - j
                nc.gpsimd.tensor_copy(
                    xp[0:kw, dst * C_:(dst + 1) * C_],
                    xp[0:kw, src * C_:(src + 1) * C_],
                )

            # ---- stage 1: contract H -> T1[wp-window][w, c*120 + (s,ho)] ----
            t1 = []
            for tp in range(NT):
                _, w0, ww = _tile_params(tp)
                t1t = t1_pool.tile([128, C_ * WSTRIDE], f32, tag=f"t1_{tp}")
                for cg in range(C_ // CG):
                    ps = ps1_pool.tile([128, CG * WSTRIDE], f32, tag="ps1")
                    for ci in range(CG):
                        c = cg * CG + ci
                        lhsT = xp[0:kw, :].rearrange("k (w c) -> k w c", c=C_)[
                            :, w0:w0 + ww, c
                        ]
                        nc.tensor.matmul(
                            ps[0:ww, ci * WSTRIDE:ci * WSTRIDE + ncols],
                            lhsT,
                            band_ap(kw, cnt),
                            start=True,
                            stop=True,
                        )
                    _evac(n_evac,
                        t1t[0:ww, cg * CG * WSTRIDE:(cg + 1) * CG * WSTRIDE]
                        .rearrange("p (ci x) -> p ci x", ci=CG)[:, :, 0:ncols],
                        ps[0:ww, :]
                        .rearrange("p (ci x) -> p ci x", ci=CG)[:, :, 0:ncols],
                    )
                    n_evac += 1
                t1.append(t1t)

            # ---- stage 2: contract W; interleave c; store 4 quadrants ----
            for t2 in range(NT):
                cnt2, _, kw2 = _tile_params(t2)
                ncols2 = 2 * cnt2
                stg = stg_pool.tile([128, 2 * T * C_], f32, tag="stg")
                for cg in range(C_ // CG):
                    ps2 = ps2_pool.tile([128, CG * WSTRIDE], f32, tag="ps2")
                    for ci in range(CG):
                        c = cg * CG + ci
                        lhsT = t1[t2][0:kw2, c * WSTRIDE:c * WSTRIDE + ncols]
                        nc.tensor.matmul(
                            ps2[0:ncols, ci * WSTRIDE:ci * WSTRIDE + ncols2],
                            lhsT,
                            band_ap(kw2, cnt2),
                            start=True,
                            stop=True,
                        )
                    # [p, ci, s', j] -> stg[p, s'*960 + j*16 + (cg*4+ci)]
                    src = (
                        ps2[0:ncols, :]
                        .rearrange("p (ci x) -> p ci x", ci=CG)[:, :, 0:ncols2]
                        .rearrange("p ci (s j) -> p ci s j", s=2)
                    )
                    dst = (
                        stg[0:ncols, :]
                        .rearrange("p (s j c) -> p s j c", s=2, j=T)[
                            :, :, 0:cnt2, cg * CG:(cg + 1) * CG
                        ]
                        .transpose([0, 3, 1, 2])
                    )
                    _evac(n_evac, dst, src)
                    n_evac += 1
                for s in range(2):
                    for sp in range(2):
                        nc.sync.dma_start(
                            outs[(s, sp)][
                                t * T:t * T + cnt, t2 * T:t2 * T + cnt2, :
                            ].rearrange("h w c -> h (w c)"),
                            stg[s * cnt:(s + 1) * cnt,
                                sp * T * C_:sp * T * C_ + cnt2 * C_],
                        )

    nc.compile()
    return nc


_NC = None


def _get_nc():
    global _NC
    if _NC is None:
        _NC = build_nc()
    return _NC


def kernel(x, dec_lo):
    from concourse import bass_utils

    x = np.ascontiguousarray(np.asarray(x, np.float32))
    band = band_matrix(dec_lo)
    nc = _get_nc()
    in_maps = [{"x": x[i], "band": band} for i in range(N_CORES)]
    res = bass_utils.run_bass_kernel_spmd(nc, in_maps, core_ids=list(range(N_CORES)))
    names = ["aa", "ad", "da", "dd"]
    return tuple(
        np.stack([res.results[i][n] for i in range(N_CORES)], axis=0) for n in names
    )
